# revision 5
# baseline (speedup 1.0000x reference)
"""Dehazing kernel for Trainium2 (Bass/Tile), 8-core data-parallel.

Problem: img [32,3,512,512] f32, w [32] f32.
  dc = 15x15 box-mean of per-pixel channel-min (zero-padded)
  A  = per-channel mean of img at top-5% dc positions (k=13107)
  t  = max(1 - w*dc, 0.1); out = clip((img-A)/(t+0.001) + A, 0, 1)

Sharding: batch dim across 8 cores, 4 images per core. Each image:
  - channel-min on DVE
  - horizontal 15-tap box sum: DVE prefix-scan + shifted subtract
  - vertical 15-tap box sum: PE banded-matrix matmuls (band consts as input)
  - top-k threshold: bisection with fused is_ge+accum counts (DVE),
    cross-partition count reduce via ones-matmul broadcast trick (PE)
  - masked channel sums: fused scalar_tensor_tensor with accum
  - dehaze: fused DVE ops + Relu on ACT + min-clamp on GPSIMD
"""
import os
import numpy as np

import concourse.bacc as bacc
import concourse.tile as tile
import concourse.mybir as mybir
from concourse.bass_utils import run_bass_kernel_spmd

F32 = mybir.dt.float32
U32 = mybir.dt.uint32
ALU = mybir.AluOpType
ACTF = mybir.ActivationFunctionType

P = 128
H = W = 512
G = H // P              # 4 row-groups
NPC = 4                 # images per core
NCORES = 8
K = 13107               # int(512*512*0.05)
KF = float(K)
KSUB = KF / G           # bisection target on the 1/4 subsample
SUB_ROUNDS = 11
FULL_ROUNDS = 13

LAST_RESULT = None


def _make_consts() -> np.ndarray:
    k = np.arange(P)[:, None]
    m = np.arange(P)[None, :]
    bdiag = (np.abs(k - m) <= 7).astype(np.float32)
    bup = ((k - m) >= 121).astype(np.float32)      # chunk g'-1 rows
    bdn = ((m - k) >= 121).astype(np.float32)      # chunk g'+1 rows
    ones = np.ones((P, P), dtype=np.float32)
    return np.concatenate([bdiag, bup, bdn, ones], axis=1)  # [128, 512]


CONSTS = _make_consts()


def _bisect_round(nc, pools, dc_view, lo, wd, kf):
    """One bisection round: tau=lo+wd/2; if count(dc>=tau)>=kf: lo=tau; wd/=2."""
    small, scratch, cnt_ps, ones_ap = pools
    tau = small.tile([P, 1], F32, tag="tau")
    nc.vector.scalar_tensor_tensor(
        out=tau[:], in0=wd[:], scalar=0.5, in1=lo[:],
        op0=ALU.mult, op1=ALU.add)
    scr = scratch.tile([P, G * W], F32, tag="scr")
    part = small.tile([P, 1], F32, tag="part")
    nc.vector.tensor_scalar(
        out=scr[:, :dc_view.free_size()], in0=dc_view, scalar1=tau[:],
        scalar2=None, op0=ALU.is_ge, op1=ALU.add, accum_out=part[:])
    cps = cnt_ps.tile([P, 1], F32, tag="cps")
    nc.tensor.matmul(cps[:], lhsT=ones_ap, rhs=part[:], start=True, stop=True)
    a = small.tile([P, 1], U32, tag="cmp")
    nc.vector.tensor_scalar(out=a[:], in0=cps[:], scalar1=kf, scalar2=None,
                            op0=ALU.is_ge)
    nc.vector.copy_predicated(lo[:], a[:], tau[:])
    nc.vector.tensor_scalar(out=wd[:], in0=wd[:], scalar1=0.5, scalar2=None,
                            op0=ALU.mult)


def _build(nc):
    img_in = nc.dram_tensor("img", [NPC, 3, H, W], F32, kind="ExternalInput").ap()
    w_in = nc.dram_tensor("w", [NPC], F32, kind="ExternalInput").ap()
    consts_in = nc.dram_tensor("consts", [P, 4 * P], F32, kind="ExternalInput").ap()
    out_d = nc.dram_tensor("out", [NPC, 3, H, W], F32, kind="ExternalOutput").ap()

    with tile.TileContext(nc) as tc:
        with (
            tc.tile_pool(name="const", bufs=1) as const_pool,
            tc.tile_pool(name="img", bufs=2) as img_pool,
            tc.tile_pool(name="dcp", bufs=2) as dc_pool,
            tc.tile_pool(name="work", bufs=2) as work,
            tc.tile_pool(name="scratch", bufs=2) as scratch,
            tc.tile_pool(name="small", bufs=6) as small,
            tc.tile_pool(name="vband", bufs=4, space="PSUM") as vband,
            tc.tile_pool(name="cntps", bufs=2, space="PSUM") as cnt_ps,
            tc.tile_pool(name="miscps", bufs=1, space="PSUM") as misc_ps,
        ):
            consts = const_pool.tile([P, 4 * P], F32)
            nc.sync.dma_start(consts[:], consts_in[:])
            bdiag = consts[:, 0:P]
            bup = consts[:, P:2 * P]
            bdn = consts[:, 2 * P:3 * P]
            ones = consts[:, 3 * P:4 * P]

            w_sb = const_pool.tile([1, NPC], F32)
            nc.sync.dma_start(w_sb[:], w_in.rearrange("(p a) -> p a", p=1))
            # broadcast w to all partitions: out[m,n] = ones[0,m] * w[0,n]
            w4_ps = misc_ps.tile([P, NPC], F32, tag="w4")
            nc.tensor.matmul(w4_ps[:], lhsT=ones[0:1, :], rhs=w_sb[:],
                             start=True, stop=True)
            negw4 = const_pool.tile([P, NPC], F32)
            nc.vector.tensor_scalar(out=negw4[:], in0=w4_ps[:], scalar1=-1.0,
                                    scalar2=None, op0=ALU.mult)

            for i in range(NPC):
                # ---- load channel planes: [128p, 4g, 512x], y = g*128+p
                imgt = []
                for c in range(3):
                    t = img_pool.tile([P, G, W], F32, tag=f"img{c}")
                    nc.sync.dma_start(
                        t[:], img_in[i, c].rearrange("(g p) x -> p g x", p=P))
                    imgt.append(t)

                # ---- channel min (GPSIMD)
                mn = work.tile([P, G, W], F32, tag="wk1")
                nc.vector.tensor_tensor(out=mn[:], in0=imgt[0][:],
                                        in1=imgt[1][:], op=ALU.min)
                nc.vector.tensor_tensor(out=mn[:], in0=mn[:], in1=imgt[2][:],
                                        op=ALU.min)

                # ---- horizontal box sum via prefix scan + shifted subtract
                Pb = work.tile([P, 2056], F32, tag="wk2")
                nc.vector.memset(Pb[:, 0:1], 0.0)
                mn_flat = mn[:].rearrange("p g x -> p (g x)")
                nc.vector.tensor_tensor_scan(
                    out=Pb[:, 1:2049], data0=mn_flat, data1=mn_flat,
                    initial=0.0, op0=ALU.add, op1=ALU.bypass)
                sh = work.tile([P, G, W], F32, tag="wk3")
                pv = Pb[:, 1:2049].rearrange("p (g x) -> p g x", g=G)
                # main region x in [8, 505): P[x+7] - P[x-8]
                nc.vector.tensor_tensor(
                    out=sh[:, :, 8:505], in0=pv[:, :, 15:512],
                    in1=pv[:, :, 0:497], op=ALU.subtract)
                for g in range(G):
                    base = g * W
                    # left edge x in [0,8): P[x+7] - rowbase
                    nc.vector.tensor_tensor(
                        out=sh[:, g, 0:8], in0=Pb[:, base + 8:base + 16],
                        in1=Pb[:, base:base + 1].to_broadcast([P, 8]),
                        op=ALU.subtract)
                    # right edge x in [505,512): P[511] - P[x-8]
                    nc.vector.tensor_tensor(
                        out=sh[:, g, 505:512],
                        in0=Pb[:, base + 512:base + 513].to_broadcast([P, 7]),
                        in1=Pb[:, base + 498:base + 505], op=ALU.subtract)

                # ---- vertical box sum via banded matmuls (PE)
                dc = dc_pool.tile([P, G, W], F32, tag="dc")
                for gp in range(G):
                    ps = vband.tile([P, W], F32, tag="vps")
                    mms = [(bdiag, gp)]
                    if gp > 0:
                        mms.append((bup, gp - 1))
                    if gp < G - 1:
                        mms.append((bdn, gp + 1))
                    for j, (band, gsrc) in enumerate(mms):
                        nc.tensor.matmul(ps[:], lhsT=band, rhs=sh[:, gsrc, :],
                                         start=(j == 0), stop=(j == len(mms) - 1))
                    nc.scalar.activation(dc[:, gp, :], ps[:], ACTF.Copy,
                                         scale=1.0 / 225.0)

                dc_flat = dc[:].rearrange("p g x -> p (g x)")

                # ---- top-k threshold by bisection
                lo = small.tile([P, 1], F32, tag="lo")
                nc.vector.memset(lo[:], 0.0)
                wd = small.tile([P, 1], F32, tag="wd")
                nc.vector.memset(wd[:], 1.01)
                pools = (small, scratch, cnt_ps, ones)
                for _ in range(SUB_ROUNDS):
                    _bisect_round(nc, pools, dc[:, 1, :], lo, wd, KSUB)
                # expand bracket to absorb subsample noise: [lo-wd, lo+2wd]
                nc.vector.tensor_tensor(out=lo[:], in0=lo[:], in1=wd[:],
                                        op=ALU.subtract)
                nc.vector.tensor_scalar(out=wd[:], in0=wd[:], scalar1=3.0,
                                        scalar2=None, op0=ALU.mult)
                for _ in range(FULL_ROUNDS):
                    _bisect_round(nc, pools, dc_flat, lo, wd, KF)

                # ---- masked count + channel sums at threshold lo
                part4 = small.tile([P, 4], F32, tag="part4")
                scrM = scratch.tile([P, G * W], F32, tag="scr")
                nc.vector.tensor_scalar(
                    out=scrM[:], in0=dc_flat, scalar1=lo[:], scalar2=None,
                    op0=ALU.is_ge, op1=ALU.add, accum_out=part4[:, 0:1])
                for c in range(3):
                    scr_c = scratch.tile([P, G * W], F32, tag="scr")
                    nc.vector.scalar_tensor_tensor(
                        out=scr_c[:], in0=dc_flat, scalar=lo[:],
                        in1=imgt[c][:].rearrange("p g x -> p (g x)"),
                        op0=ALU.is_ge, op1=ALU.mult,
                        accum_out=part4[:, c + 1:c + 2])
                tot_ps = misc_ps.tile([P, 4], F32, tag="tot")
                nc.tensor.matmul(tot_ps[:], lhsT=ones, rhs=part4[:],
                                 start=True, stop=True)
                rcount = small.tile([P, 1], F32, tag="rcount")
                nc.vector.reciprocal(out=rcount[:], in_=tot_ps[:, 0:1])
                A3 = small.tile([P, 3], F32, tag="A3")
                nc.vector.tensor_tensor(out=A3[:], in0=tot_ps[:, 1:4],
                                        in1=rcount[:].to_broadcast([P, 3]),
                                        op=ALU.mult)

                # ---- transmission map + reciprocal
                tm = work.tile([P, G * W], F32, tag="wk1")
                nc.vector.tensor_scalar(
                    out=tm[:], in0=dc_flat, scalar1=negw4[:, i:i + 1],
                    scalar2=1.0, op0=ALU.mult, op1=ALU.add)
                nc.vector.tensor_scalar(
                    out=tm[:], in0=tm[:], scalar1=0.001, scalar2=0.101,
                    op0=ALU.add, op1=ALU.max)
                rr = work.tile([P, G * W], F32, tag="wk2")
                nc.vector.reciprocal_approx_fast(out=rr[:], in_=tm[:])

                # ---- dehaze per channel, in-place into img tile, store
                for c in range(3):
                    img_flat = imgt[c][:].rearrange("p g x -> p (g x)")
                    d = work.tile([P, G * W], F32, tag="wk3")
                    nc.vector.scalar_tensor_tensor(
                        out=d[:], in0=img_flat, scalar=A3[:, c:c + 1], in1=rr[:],
                        op0=ALU.subtract, op1=ALU.mult)
                    u2 = work.tile([P, G * W], F32, tag="wk4")
                    nc.scalar.activation(u2[:], d[:], ACTF.Relu,
                                         bias=A3[:, c:c + 1], scale=1.0)
                    nc.gpsimd.tensor_scalar(out=img_flat, in0=u2[:],
                                            scalar1=1.0, scalar2=None,
                                            op0=ALU.min)
                    nc.sync.dma_start(
                        out_d[i, c].rearrange("(g p) x -> p g x", p=P),
                        imgt[c][:])
    nc.compile()
    return nc


_NC_CACHE = None


def _get_nc():
    global _NC_CACHE
    if _NC_CACHE is None:
        nc = bacc.Bacc("TRN2", target_bir_lowering=False, debug=False)
        _NC_CACHE = _build(nc)
    return _NC_CACHE


def kernel(img: np.ndarray, w: np.ndarray) -> np.ndarray:
    global LAST_RESULT
    img = np.ascontiguousarray(np.asarray(img, dtype=np.float32))
    w = np.ascontiguousarray(np.asarray(w, dtype=np.float32))
    nc = _get_nc()
    in_maps = [
        {"img": img[i * NPC:(i + 1) * NPC], "w": w[i * NPC:(i + 1) * NPC],
         "consts": CONSTS}
        for i in range(NCORES)
    ]
    trace = bool(int(os.environ.get("DEHAZE_TRACE", "0")))
    res = run_bass_kernel_spmd(nc, in_maps, list(range(NCORES)), trace=trace)
    LAST_RESULT = res
    return np.concatenate([r["out"] for r in res.results], axis=0)


# revision 7
# speedup vs baseline: 2.4313x; 2.4313x over previous
"""Dehazing kernel for Trainium2 (Bass/Tile), 8-core data-parallel.

Problem: img [32,3,512,512] f32, w [32] f32.
  dc = 15x15 box-mean of per-pixel channel-min (zero-padded)
  A  = per-channel mean of img at top-5% dc positions (k=13107)
  t  = max(1 - w*dc, 0.1); out = clip((img-A)/(t+0.001) + A, 0, 1)

Sharding: batch dim across 8 cores, 4 images per core (2 pairs). Per image:
  - channel-min on DVE
  - horizontal 15-tap box sum: DVE prefix-scan + shifted subtract
  - vertical 15-tap box sum: PE banded-matrix matmuls (band consts as input)
  - top-k threshold: pair-batched bisection; per-round counts via fused
    is_ge+accum (DVE) for image 0 and Sign+accum on ACT for image 1;
    cross-partition count reduce via ones-matmul broadcast trick (PE)
  - masked channel sums: fused scalar_tensor_tensor with accum
  - dehaze: fused DVE ops + Relu on ACT
"""
import os
import numpy as np

import concourse.bacc as bacc
import concourse.tile as tile
import concourse.mybir as mybir
from concourse.bass_utils import run_bass_kernel_spmd

F32 = mybir.dt.float32
U32 = mybir.dt.uint32
ALU = mybir.AluOpType
ACTF = mybir.ActivationFunctionType

P = 128
H = W = 512
G = H // P              # 4 row-groups
NPC = 4                 # images per core
NCORES = 8
K = 13107               # int(512*512*0.05)
KF = float(K)
NFULL = float(H * W)
NS = float(P * W)       # subsample size (one row-group)
KS = KF / G             # bisection target on the 1/4 subsample
SUB_ROUNDS = 11
FULL_ROUNDS = 13

LAST_RESULT = None


def _make_consts() -> np.ndarray:
    k = np.arange(P)[:, None]
    m = np.arange(P)[None, :]
    bdiag = (np.abs(k - m) <= 7).astype(np.float32)
    bup = ((k - m) >= 121).astype(np.float32)      # chunk g'-1 rows
    bdn = ((m - k) >= 121).astype(np.float32)      # chunk g'+1 rows
    ones = np.ones((P, P), dtype=np.float32)
    return np.concatenate([bdiag, bup, bdn, ones], axis=1)  # [128, 512]


CONSTS = _make_consts()


def _build(nc):
    img_in = nc.dram_tensor("img", [NPC, 3, H, W], F32, kind="ExternalInput").ap()
    w_in = nc.dram_tensor("w", [NPC], F32, kind="ExternalInput").ap()
    consts_in = nc.dram_tensor("consts", [P, 4 * P], F32, kind="ExternalInput").ap()
    out_d = nc.dram_tensor("out", [NPC, 3, H, W], F32, kind="ExternalOutput").ap()

    with tile.TileContext(nc) as tc:
        with (
            tc.tile_pool(name="const", bufs=1) as const_pool,
            tc.tile_pool(name="img", bufs=3) as img_pool,
            tc.tile_pool(name="dcp", bufs=3) as dc_pool,
            tc.tile_pool(name="work", bufs=2) as work,
            tc.tile_pool(name="scratch", bufs=2) as scratch,
            tc.tile_pool(name="small", bufs=6) as small,
            tc.tile_pool(name="vband", bufs=4, space="PSUM") as vband,
            tc.tile_pool(name="cntps", bufs=2, space="PSUM") as cnt_ps,
            tc.tile_pool(name="miscps", bufs=1, space="PSUM") as misc_ps,
        ):
            consts = const_pool.tile([P, 4 * P], F32)
            nc.sync.dma_start(consts[:], consts_in[:])
            bdiag = consts[:, 0:P]
            bup = consts[:, P:2 * P]
            bdn = consts[:, 2 * P:3 * P]
            ones = consts[:, 3 * P:4 * P]

            # bisection compare thresholds per pair-column:
            #   col0 (DVE is_ge count): count >= k
            #   col1 (ACT sign-sum):    sum >= 2k - N
            kvec_sub = const_pool.tile([P, 2], F32)
            nc.vector.memset(kvec_sub[:, 0:1], KS)
            nc.vector.memset(kvec_sub[:, 1:2], 2.0 * KS - NS)
            kvec_full = const_pool.tile([P, 2], F32)
            nc.vector.memset(kvec_full[:, 0:1], KF)
            nc.vector.memset(kvec_full[:, 1:2], 2.0 * KF - NFULL)

            w_sb = const_pool.tile([1, NPC], F32)
            nc.sync.dma_start(w_sb[:], w_in.rearrange("(p a) -> p a", p=1))
            # broadcast w to all partitions: out[m,n] = ones[0,m] * w[0,n]
            w4_ps = misc_ps.tile([P, NPC], F32, tag="w4")
            nc.tensor.matmul(w4_ps[:], lhsT=ones[0:1, :], rhs=w_sb[:],
                             start=True, stop=True)
            negw4 = const_pool.tile([P, NPC], F32)
            nc.vector.tensor_scalar(out=negw4[:], in0=w4_ps[:], scalar1=-1.0,
                                    scalar2=None, op0=ALU.mult)

            def phase1(i):
                """load + channel-min + box filter -> (img tiles, dc tile)"""
                imgt = []
                for c in range(3):
                    t = img_pool.tile([P, G, W], F32, tag=f"img{c}")
                    nc.sync.dma_start(
                        t[:], img_in[i, c].rearrange("(g p) x -> p g x", p=P))
                    imgt.append(t)

                mn = work.tile([P, G, W], F32, tag="wk1")
                nc.vector.tensor_tensor(out=mn[:], in0=imgt[0][:],
                                        in1=imgt[1][:], op=ALU.min)
                nc.vector.tensor_tensor(out=mn[:], in0=mn[:], in1=imgt[2][:],
                                        op=ALU.min)

                # horizontal box sum via prefix scan + shifted subtract
                Pb = work.tile([P, 2056], F32, tag="wk2")
                nc.vector.memset(Pb[:, 0:1], 0.0)
                mn_flat = mn[:].rearrange("p g x -> p (g x)")
                nc.vector.tensor_tensor_scan(
                    out=Pb[:, 1:2049], data0=mn_flat, data1=mn_flat,
                    initial=0.0, op0=ALU.add, op1=ALU.bypass)
                # sh overwrites mn's storage (mn dead after the scan)
                sh = mn
                pv = Pb[:, 1:2049].rearrange("p (g x) -> p g x", g=G)
                nc.vector.tensor_tensor(
                    out=sh[:, :, 8:505], in0=pv[:, :, 15:512],
                    in1=pv[:, :, 0:497], op=ALU.subtract)
                for g in range(G):
                    base = g * W
                    nc.vector.tensor_tensor(
                        out=sh[:, g, 0:8], in0=Pb[:, base + 8:base + 16],
                        in1=Pb[:, base:base + 1].to_broadcast([P, 8]),
                        op=ALU.subtract)
                    nc.vector.tensor_tensor(
                        out=sh[:, g, 505:512],
                        in0=Pb[:, base + 512:base + 513].to_broadcast([P, 7]),
                        in1=Pb[:, base + 498:base + 505], op=ALU.subtract)

                # vertical box sum via banded matmuls (PE), scale 1/225 on copy
                dc = dc_pool.tile([P, G, W], F32, tag="dc")
                for gp in range(G):
                    ps = vband.tile([P, W], F32, tag="vps")
                    mms = [(bdiag, gp)]
                    if gp > 0:
                        mms.append((bup, gp - 1))
                    if gp < G - 1:
                        mms.append((bdn, gp + 1))
                    for j, (band, gsrc) in enumerate(mms):
                        nc.tensor.matmul(ps[:], lhsT=band, rhs=sh[:, gsrc, :],
                                         start=(j == 0), stop=(j == len(mms) - 1))
                    nc.scalar.activation(dc[:, gp, :], ps[:], ACTF.Copy,
                                         scale=1.0 / 225.0)
                return imgt, dc

            def bisect_rounds(views, lo2, wd2, kvec, rounds):
                fs = views[0].free_size()
                for _ in range(rounds):
                    tau2 = small.tile([P, 2], F32, tag="tau")
                    nc.vector.scalar_tensor_tensor(
                        out=tau2[:], in0=wd2[:], scalar=0.5, in1=lo2[:],
                        op0=ALU.mult, op1=ALU.add)
                    ntau = small.tile([P, 1], F32, tag="ntau")
                    nc.vector.tensor_scalar(
                        out=ntau[:], in0=tau2[:, 1:2], scalar1=-1.0,
                        scalar2=None, op0=ALU.mult)
                    part2 = small.tile([P, 2], F32, tag="part")
                    scr = scratch.tile([P, G * W], F32, tag="scr")
                    nc.vector.tensor_scalar(
                        out=scr[:, :fs], in0=views[0], scalar1=tau2[:, 0:1],
                        scalar2=None, op0=ALU.is_ge, op1=ALU.add,
                        accum_out=part2[:, 0:1])
                    scr2 = scratch.tile([P, G * W], F32, tag="scr")
                    nc.scalar.activation(
                        scr2[:, :fs], views[1], ACTF.Sign, bias=ntau[:],
                        scale=1.0, accum_out=part2[:, 1:2])
                    cps = cnt_ps.tile([P, 2], F32, tag="cps")
                    nc.tensor.matmul(cps[:], lhsT=ones, rhs=part2[:],
                                     start=True, stop=True)
                    a2 = small.tile([P, 2], U32, tag="cmp")
                    nc.vector.tensor_tensor(out=a2[:], in0=cps[:], in1=kvec[:],
                                            op=ALU.is_ge)
                    nc.vector.copy_predicated(lo2[:], a2[:], tau2[:])
                    nc.vector.tensor_scalar(out=wd2[:], in0=wd2[:], scalar1=0.5,
                                            scalar2=None, op0=ALU.mult)

            def finals(i, imgt, dc, lo):
                """masked sums, A, t-map, dehaze, store. lo: [P,1] AP."""
                dc_flat = dc[:].rearrange("p g x -> p (g x)")
                part4 = small.tile([P, 4], F32, tag="part4")
                scrM = scratch.tile([P, G * W], F32, tag="scr")
                nc.vector.tensor_scalar(
                    out=scrM[:], in0=dc_flat, scalar1=lo, scalar2=None,
                    op0=ALU.is_ge, op1=ALU.add, accum_out=part4[:, 0:1])
                for c in range(3):
                    scr_c = scratch.tile([P, G * W], F32, tag="scr")
                    nc.vector.scalar_tensor_tensor(
                        out=scr_c[:], in0=dc_flat, scalar=lo,
                        in1=imgt[c][:].rearrange("p g x -> p (g x)"),
                        op0=ALU.is_ge, op1=ALU.mult,
                        accum_out=part4[:, c + 1:c + 2])
                tot_ps = misc_ps.tile([P, 4], F32, tag="tot")
                nc.tensor.matmul(tot_ps[:], lhsT=ones, rhs=part4[:],
                                 start=True, stop=True)
                rcount = small.tile([P, 1], F32, tag="rcount")
                nc.vector.reciprocal(out=rcount[:], in_=tot_ps[:, 0:1])
                A3 = small.tile([P, 3], F32, tag="A3")
                nc.vector.tensor_tensor(out=A3[:], in0=tot_ps[:, 1:4],
                                        in1=rcount[:].to_broadcast([P, 3]),
                                        op=ALU.mult)

                tm = work.tile([P, G * W], F32, tag="wk1")
                nc.vector.tensor_scalar(
                    out=tm[:], in0=dc_flat, scalar1=negw4[:, i:i + 1],
                    scalar2=1.0, op0=ALU.mult, op1=ALU.add)
                nc.vector.tensor_scalar(
                    out=tm[:], in0=tm[:], scalar1=0.001, scalar2=0.101,
                    op0=ALU.add, op1=ALU.max)
                rr = work.tile([P, G * W], F32, tag="wk2")
                nc.vector.reciprocal_approx_fast(out=rr[:], in_=tm[:])

                for c in range(3):
                    img_flat = imgt[c][:].rearrange("p g x -> p (g x)")
                    d = work.tile([P, G * W], F32, tag="wk3")
                    nc.vector.scalar_tensor_tensor(
                        out=d[:], in0=img_flat, scalar=A3[:, c:c + 1], in1=rr[:],
                        op0=ALU.subtract, op1=ALU.mult)
                    u2 = work.tile([P, G * W], F32, tag="wk4")
                    nc.scalar.activation(u2[:], d[:], ACTF.Relu,
                                         bias=A3[:, c:c + 1], scale=1.0)
                    nc.vector.tensor_scalar(out=img_flat, in0=u2[:],
                                            scalar1=1.0, scalar2=None,
                                            op0=ALU.min)
                    nc.sync.dma_start(
                        out_d[i, c].rearrange("(g p) x -> p g x", p=P),
                        imgt[c][:])

            for pair in range(NPC // 2):
                ia, ib = 2 * pair, 2 * pair + 1
                imgt_a, dc_a = phase1(ia)
                imgt_b, dc_b = phase1(ib)
                lo2 = small.tile([P, 2], F32, tag="lo")
                nc.vector.memset(lo2[:], 0.0)
                wd2 = small.tile([P, 2], F32, tag="wd")
                nc.vector.memset(wd2[:], 1.01)
                bisect_rounds([dc_a[:, 1, :], dc_b[:, 1, :]], lo2, wd2,
                              kvec_sub, SUB_ROUNDS)
                # expand bracket: [lo-wd, lo+2wd] absorbs subsample noise
                nc.vector.tensor_tensor(out=lo2[:], in0=lo2[:], in1=wd2[:],
                                        op=ALU.subtract)
                nc.vector.tensor_scalar(out=wd2[:], in0=wd2[:], scalar1=3.0,
                                        scalar2=None, op0=ALU.mult)
                bisect_rounds([dc_a[:].rearrange("p g x -> p (g x)"),
                               dc_b[:].rearrange("p g x -> p (g x)")],
                              lo2, wd2, kvec_full, FULL_ROUNDS)
                finals(ia, imgt_a, dc_a, lo2[:, 0:1])
                finals(ib, imgt_b, dc_b, lo2[:, 1:2])
    nc.compile()
    return nc


_NC_CACHE = None


def _get_nc():
    global _NC_CACHE
    if _NC_CACHE is None:
        nc = bacc.Bacc("TRN2", target_bir_lowering=False, debug=False)
        _NC_CACHE = _build(nc)
    return _NC_CACHE


def kernel(img: np.ndarray, w: np.ndarray) -> np.ndarray:
    global LAST_RESULT
    img = np.ascontiguousarray(np.asarray(img, dtype=np.float32))
    w = np.ascontiguousarray(np.asarray(w, dtype=np.float32))
    nc = _get_nc()
    in_maps = [
        {"img": img[i * NPC:(i + 1) * NPC], "w": w[i * NPC:(i + 1) * NPC],
         "consts": CONSTS}
        for i in range(NCORES)
    ]
    trace = bool(int(os.environ.get("DEHAZE_TRACE", "0")))
    res = run_bass_kernel_spmd(nc, in_maps, list(range(NCORES)), trace=trace)
    LAST_RESULT = res
    return np.concatenate([r["out"] for r in res.results], axis=0)
